# revision 1
# baseline (speedup 1.0000x reference)
"""Binary (sign-quantized weight) 3x3 conv, stride 1, pad 1, on 8 trn2 cores.

Problem: x[32,128,56,56] f32, weight[256,128,3,3] f32, bias[256] f32
         y = conv2d(x, sign(weight), pad=1) + bias      -> [32,256,56,56] f32

Strategy:
  - Data-parallel over batch: 4 images per core, weight/bias replicated.
  - Per core: x is loaded per-image as [ci=128 partitions, 56*56] f32 and
    cast to bf16 (unit stride, no physical padding). The 3x3 conv is 9
    shifted [128ci -> 128co] matmuls accumulated in PSUM per output tile
    of 8 rows x 56 cols (N=448). Padding is implicit: boundary taps use
    narrowed row/col ranges (PSUM per-element has_written gives
    overwrite-on-first-write, so partial-coverage accumulation is exact).
  - Weights are host-relaid to [ci, (kh kw co)] f32; sign+cast to bf16 on
    device (ScalarE). bf16 is exact for {-1,0,1}; x bf16 rounding gives
    ~2e-3 rel error. PSUM accumulates in f32.
  - Output tiles [co=128, 448] get bias added on VectorE on the way out
    (PSUM -> SBUF f32), then DMA to DRAM.
  - Warm-up: dummy sign op preloads the ACT table; zero matmuls keep the
    PE busy from t~0 so the HAM clock gate is at full speed when real
    matmuls start.
"""

import sys

sys.path.insert(0, "/opt/trn_rl_repo")

from contextlib import ExitStack

import numpy as np

B, CI, CO, KK, H, W = 32, 128, 256, 3, 56, 56
N_CORES = 8
B_SH = B // N_CORES  # 4 images per core
ROWS_PER_MM = 8  # output rows per matmul -> N = 448 <= 512 (one PSUM bank)
N_MM = ROWS_PER_MM * W  # 448
N_RB = H // ROWS_PER_MM  # 7 row blocks

_NC_CACHE = None


def _build():
    import concourse.tile as tile
    from concourse import bacc, mybir

    nc = bacc.Bacc("TRN2", target_bir_lowering=False, debug=False)

    x_d = nc.dram_tensor("x", [B_SH, CI, H, W], mybir.dt.float32, kind="ExternalInput")
    wt_d = nc.dram_tensor(
        "wt", [CI, KK * KK * CO], mybir.dt.bfloat16, kind="ExternalInput"
    )
    b_d = nc.dram_tensor("bias2", [128, CO // 128], mybir.dt.float32, kind="ExternalInput")
    y_d = nc.dram_tensor("y", [B_SH, CO, H, W], mybir.dt.float32, kind="ExternalOutput")

    x_ap = x_d.ap().rearrange("b c h w -> b c (h w)")
    y_ap = y_d.ap().rearrange("b c h w -> b c (h w)")
    x_img = x_d.ap()  # [b, c, h, w]

    with tile.TileContext(nc) as tc:
        with ExitStack() as ctx:
            singles = ctx.enter_context(tc.tile_pool(name="singles", bufs=1))
            xf_pool = ctx.enter_context(tc.tile_pool(name="xf", bufs=3))
            xb_pool = ctx.enter_context(tc.tile_pool(name="xb", bufs=3))
            ps_pool = ctx.enter_context(
                tc.tile_pool(name="ps", bufs=8, space="PSUM")
            )
            yo_pool = ctx.enter_context(tc.tile_pool(name="yo", bufs=6))

            wt_ap = wt_d.ap().rearrange("p (t c) -> p t c", c=CO)
            w_bin = singles.tile([CI, KK * KK, CO], mybir.dt.bfloat16)

            # PE warm-up: zero matmuls so the HAM clock gate (and the cost
            # model's p-state ramp) is at full speed when real matmuls begin
            warm_w = singles.tile([128, 128], mybir.dt.bfloat16)
            warm_x = singles.tile([128, N_MM], mybir.dt.bfloat16)
            nc.vector.memset(warm_w[:, :], 0.0)
            nc.vector.memset(warm_x[:, :], 0.0)
            for _ in range(6):
                warm_ps = ps_pool.tile([128, N_MM], mybir.dt.float32, tag="ps")
                nc.tensor.matmul(
                    warm_ps[:, :], warm_w[:, :], warm_x[:, :], start=True, stop=True
                )

            def load_chunk(xf3, xb3, b, c):
                r0 = c * ROWS_PER_MM
                nc.sync.dma_start(
                    out=xf3[:, r0 : r0 + ROWS_PER_MM, :],
                    in_=x_img[b, :, r0 : r0 + ROWS_PER_MM, :],
                )
                nc.vector.tensor_copy(
                    out=xb3[:, r0 : r0 + ROWS_PER_MM, :],
                    in_=xf3[:, r0 : r0 + ROWS_PER_MM, :],
                )

            def alloc_img():
                xf = xf_pool.tile([CI, H * W], mybir.dt.float32, tag="xf")
                xb = xb_pool.tile([CI, H * W], mybir.dt.bfloat16, tag="xb")
                return (
                    xf.rearrange("p (h w) -> p h w", w=W),
                    xb.rearrange("p (h w) -> p h w", w=W),
                )

            def load_tap(t):
                nc.sync.dma_start(out=w_bin[:, t, :], in_=wt_ap[:, t, :])

            # startup-critical order: b=0 chunks and early taps first
            warm_a = singles.tile([128, 1], mybir.dt.float32)
            nc.vector.memset(warm_a[:, :], 0.0)
            nc.scalar.activation(
                warm_a[:, :], warm_a[:, :], mybir.ActivationFunctionType.Identity
            )
            xf3_0, xb3_0 = alloc_img()
            nc.sync.dma_start(
                out=xf3_0[:, 0:ROWS_PER_MM, :], in_=x_img[0, :, 0:ROWS_PER_MM, :]
            )
            nc.vector.tensor_copy(out=xb3_0[:, 0:4, :], in_=xf3_0[:, 0:4, :])
            nc.scalar.activation(
                xb3_0[:, 4:8, :], xf3_0[:, 4:8, :],
                mybir.ActivationFunctionType.Identity,
            )
            nc.sync.dma_start(out=w_bin[:, 0:3, :], in_=wt_ap[:, 0:3, :])
            load_chunk(xf3_0, xb3_0, 0, 1)
            nc.sync.dma_start(out=w_bin[:, 3:6, :], in_=wt_ap[:, 3:6, :])
            load_chunk(xf3_0, xb3_0, 0, 2)
            nc.sync.dma_start(out=w_bin[:, 6:9, :], in_=wt_ap[:, 6:9, :])
            for c in range(3, N_RB):
                load_chunk(xf3_0, xb3_0, 0, c)
            bias_sb = singles.tile([128, CO // 128], mybir.dt.float32)
            nc.sync.dma_start(out=bias_sb[:, :], in_=b_d.ap())

            for b in range(B_SH):
                if b == 0:
                    xb3 = xb3_0
                else:
                    xf3, xb3 = alloc_img()
                    for c in range(N_RB):
                        load_chunk(xf3, xb3, b, c)

                for rb in range(N_RB):
                    for c2 in range(CO // 128):
                        r0 = rb * ROWS_PER_MM
                        ps = ps_pool.tile([128, N_MM], mybir.dt.float32, tag="ps")
                        ps3 = ps.rearrange("p (r w) -> p r w", w=W)
                        i = 0
                        for kh in range(KK):
                            # output rows (within block) whose input row is
                            # in [0, H)
                            a = max(0, (1 - kh) - r0)
                            bb = min(ROWS_PER_MM, (H + 1) - kh - r0)
                            for kw in range(KK):
                                c0 = max(0, 1 - kw)
                                c1 = W - max(0, kw - 1)
                                rhs = xb3[
                                    :,
                                    r0 + a + kh - 1 : r0 + bb + kh - 1,
                                    c0 + kw - 1 : c1 + kw - 1,
                                ]
                                lhsT = w_bin[:, kh * KK + kw, c2 * 128 : (c2 + 1) * 128]
                                nc.tensor.matmul(
                                    ps3[:, a:bb, c0:c1],
                                    lhsT,
                                    rhs,
                                    start=(i == 0),
                                    stop=(i == KK * KK - 1),
                                    skip_group_check=True,
                                )
                                i += 1
                        ys = yo_pool.tile([128, N_MM], mybir.dt.float32, tag="ys")
                        nc.vector.tensor_scalar_add(
                            ys[:, :], ps[:, :], bias_sb[:, c2 : c2 + 1]
                        )
                        nc.sync.dma_start(
                            out=y_ap[
                                b,
                                c2 * 128 : (c2 + 1) * 128,
                                rb * N_MM : (rb + 1) * N_MM,
                            ],
                            in_=ys[:, :],
                        )
    nc.compile()
    return nc


def _get_nc():
    global _NC_CACHE
    if _NC_CACHE is None:
        _NC_CACHE = _build()
    return _NC_CACHE


def kernel(x, weight, bias):
    from concourse.bass_utils import run_bass_kernel_spmd

    x = np.ascontiguousarray(np.asarray(x, dtype=np.float32))
    weight = np.asarray(weight, dtype=np.float32)
    bias = np.asarray(bias, dtype=np.float32)

    import ml_dtypes

    # binarize on host (sharding hint: "replicate the small binarized
    # weight"); {-1,0,1} is exact in bf16. [co,ci,kh,kw] -> [ci,(kh kw co)]
    wt = np.ascontiguousarray(
        np.sign(weight).transpose(1, 2, 3, 0).reshape(CI, KK * KK * CO)
    ).astype(ml_dtypes.bfloat16)
    # bias2[p, c2] = bias[c2*128 + p]
    bias2 = np.ascontiguousarray(bias.reshape(CO // 128, 128).T)

    nc = _get_nc()
    in_maps = [
        {"x": x[i * B_SH : (i + 1) * B_SH], "wt": wt, "bias2": bias2}
        for i in range(N_CORES)
    ]
    res = run_bass_kernel_spmd(nc, in_maps, core_ids=list(range(N_CORES)))
    return np.concatenate([r["y"] for r in res.results], axis=0)



# revision 2
# speedup vs baseline: 1.7192x; 1.7192x over previous
"""Binary (sign-quantized weight) 3x3 conv, stride 1, pad 1, on 8 trn2 cores.

Problem: x[32,128,56,56] f32, weight[256,128,3,3] f32, bias[256] f32
         y = conv2d(x, sign(weight), pad=1) + bias      -> [32,256,56,56] f32

Strategy (v2 — fp8 DoubleRow):
  - Data-parallel over batch: 4 images per core, weight/bias replicated.
  - x is split on host into hi = fp8_e4m3(x) and lo = fp8_e4m3(x - hi);
    conv(x) = conv(hi) + conv(lo) up to ~0.3% error. The two planes ride
    in the two K-slots of a DoubleRow fp8 matmul (K=256 per pass, 0.5
    PE cycles per output column -> 2x bf16 throughput), with the sign
    weights duplicated across both slots.
  - Each image is zero-padded on host to 58x58 (plus a 2-byte plane
    tail). Every tap (kh,kw) of an 8-row output block is then ONE
    contiguous 464-element span at offset (r0+kh)*58+kw: no boundary
    narrowing anywhere. Column wrap pollutes only padded output columns
    56/57, which are never drained.
  - Per output tile [co=128 x (8 rows x 58)]: 9 DoubleRow matmuls
    accumulate in one PSUM bank; DVE adds bias and writes bf16 to SBUF
    reading only the 56 real columns; DMA out bf16; host upcasts to f32.
  - Warm-up zero matmuls keep the PE p-state ramp hot while the first
    weight/x DMAs land.
"""

import sys

sys.path.insert(0, "/opt/trn_rl_repo")

from contextlib import ExitStack

import numpy as np

B, CI, CO, KK, H, W = 32, 128, 256, 3, 56, 56
N_CORES = 8
B_SH = B // N_CORES  # 4 images per core
HP, WP = H + 2, W + 2  # zero-padded image
PLANE = HP * WP + 2  # +2 tail pad: tap (kh=2,kw=2) span overruns by kw
ROWS = 8  # output rows per PSUM tile
N_MM = ROWS * WP  # 464 <= 512 (one PSUM bank)
N_RB = H // ROWS  # 7 row blocks
N_WARM = 10

_NC_CACHE = None


def _build():
    import concourse.tile as tile
    from concourse import bacc, mybir

    nc = bacc.Bacc("TRN2", target_bir_lowering=False, debug=False)

    x_d = nc.dram_tensor(
        "x8", [B_SH, CI, 2 * PLANE], mybir.dt.float8e4, kind="ExternalInput"
    )
    wt_d = nc.dram_tensor(
        "wt", [CI, KK * KK * 2 * CO], mybir.dt.float8e4, kind="ExternalInput"
    )
    b_d = nc.dram_tensor(
        "bias2", [128, CO // 128], mybir.dt.float32, kind="ExternalInput"
    )
    y_d = nc.dram_tensor("y", [B_SH, CO, H * W], mybir.dt.bfloat16, kind="ExternalOutput")

    x_ap = x_d.ap().rearrange("b p (s n) -> b p s n", s=2)  # [b, ci, slot, PLANE]
    y_ap = y_d.ap()

    with tile.TileContext(nc) as tc:
        with ExitStack() as ctx:
            singles = ctx.enter_context(tc.tile_pool(name="singles", bufs=1))
            x_pool = ctx.enter_context(tc.tile_pool(name="xp", bufs=2))
            ps_pool = ctx.enter_context(tc.tile_pool(name="ps", bufs=8, space="PSUM"))
            yo_pool = ctx.enter_context(tc.tile_pool(name="yo", bufs=6))

            w_bin = singles.tile([CI, KK * KK * 2 * CO], mybir.dt.float8e4)
            w4 = w_bin.rearrange("p (t s c) -> p t s c", t=KK * KK, s=2)

            # PE warm-up: zero matmuls so the p-state ramp is at full speed
            # (and uninterrupted) by the time real matmuls begin
            warm_w = singles.tile([128, 128], mybir.dt.bfloat16)
            warm_x = singles.tile([128, N_MM], mybir.dt.bfloat16)
            nc.vector.memset(warm_w[:, :], 0.0)
            nc.vector.memset(warm_x[:, :], 0.0)
            for _ in range(N_WARM):
                warm_ps = ps_pool.tile([128, N_MM], mybir.dt.float32, tag="ps")
                nc.tensor.matmul(
                    warm_ps[:, :], warm_w[:, :], warm_x[:, :], start=True, stop=True
                )

            def alloc_img():
                xt = x_pool.tile([CI, 2 * PLANE], mybir.dt.float8e4, tag="xt")
                return xt.rearrange("p (s n) -> p s n", s=2)

            # startup-critical order: kh=0 taps + x0 top rows first
            wt_ap = wt_d.ap()
            R0 = 30 * WP  # rows [0,30) of each plane
            nc.sync.dma_start(out=w_bin[:, 0 : 3 * 2 * CO], in_=wt_ap[:, 0 : 3 * 2 * CO])
            xt0 = alloc_img()
            nc.sync.dma_start(out=xt0[:, :, 0:R0], in_=x_ap[0, :, :, 0:R0])
            nc.sync.dma_start(
                out=w_bin[:, 3 * 2 * CO :], in_=wt_ap[:, 3 * 2 * CO :]
            )
            nc.sync.dma_start(out=xt0[:, :, R0:PLANE], in_=x_ap[0, :, :, R0:PLANE])
            bias_sb = singles.tile([128, CO // 128], mybir.dt.float32)
            nc.sync.dma_start(out=bias_sb[:, :], in_=b_d.ap())

            for b in range(B_SH):
                if b == 0:
                    xt = xt0
                else:
                    xt = alloc_img()
                    nc.sync.dma_start(out=xt[:, :, :], in_=x_ap[b, :, :, :])

                for rb in range(N_RB):
                    r0 = rb * ROWS
                    for c2 in range(CO // 128):
                        ps = ps_pool.tile([128, N_MM], mybir.dt.float32, tag="ps")
                        for t in range(KK * KK):
                            kh, kw = t // KK, t % KK
                            base = (r0 + kh) * WP + kw
                            nc.tensor.matmul(
                                ps[:, :],
                                w4[:, t, :, c2 * 128 : (c2 + 1) * 128],
                                xt[:, :, base : base + N_MM],
                                start=(t == 0),
                                stop=(t == KK * KK - 1),
                                perf_mode=mybir.MatmulPerfMode.DoubleRow,
                                skip_group_check=True,
                            )
                        ps3 = ps.rearrange("p (r w) -> p r w", w=WP)
                        ys = yo_pool.tile([128, ROWS * W], mybir.dt.bfloat16, tag="ys")
                        ys3 = ys.rearrange("p (r w) -> p r w", w=W)
                        nc.vector.tensor_scalar_add(
                            ys3[:, :, :], ps3[:, :, 0:W], bias_sb[:, c2 : c2 + 1]
                        )
                        nc.sync.dma_start(
                            out=y_ap[
                                b,
                                c2 * 128 : (c2 + 1) * 128,
                                r0 * W : (r0 + ROWS) * W,
                            ],
                            in_=ys[:, :],
                        )
    nc.compile()
    return nc


def _get_nc():
    global _NC_CACHE
    if _NC_CACHE is None:
        _NC_CACHE = _build()
    return _NC_CACHE


def kernel(x, weight, bias):
    from concourse.bass_utils import run_bass_kernel_spmd

    import ml_dtypes

    f8 = ml_dtypes.float8_e4m3
    x = np.asarray(x, dtype=np.float32)
    weight = np.asarray(weight, dtype=np.float32)
    bias = np.asarray(bias, dtype=np.float32)

    # hi/lo fp8 residual split of x, zero-padded to 58x58 (+2 tail)
    hi = x.astype(f8)
    lo = (x - hi.astype(np.float32)).astype(f8)
    x8 = np.zeros((B, CI, 2, HP, WP), dtype=f8)
    x8[:, :, 0, 1 : H + 1, 1 : W + 1] = hi
    x8[:, :, 1, 1 : H + 1, 1 : W + 1] = lo
    x8p = np.zeros((B, CI, 2, PLANE), dtype=f8)
    x8p[:, :, :, : HP * WP] = x8.reshape(B, CI, 2, HP * WP)
    x8p = np.ascontiguousarray(x8p.reshape(B, CI, 2 * PLANE))

    # [co,ci,kh,kw] -> [ci, (tap slot co)], sign duplicated in both slots
    ws = np.sign(weight).transpose(1, 2, 3, 0).reshape(CI, KK * KK, 1, CO)
    wt = np.ascontiguousarray(
        np.broadcast_to(ws, (CI, KK * KK, 2, CO)).reshape(CI, KK * KK * 2 * CO)
    ).astype(f8)
    # bias2[p, c2] = bias[c2*128 + p]
    bias2 = np.ascontiguousarray(bias.reshape(CO // 128, 128).T)

    nc = _get_nc()
    in_maps = [
        {"x8": x8p[i * B_SH : (i + 1) * B_SH], "wt": wt, "bias2": bias2}
        for i in range(N_CORES)
    ]
    res = run_bass_kernel_spmd(nc, in_maps, core_ids=list(range(N_CORES)))
    y = np.concatenate([np.asarray(r["y"]) for r in res.results], axis=0)
    return y.astype(np.float32).reshape(B, CO, H, W)
